# revision 22
# baseline (speedup 1.0000x reference)
"""Trainium2 Bass kernel for nn_CriticNetwork (3x GATConv + pool + MLP head).

Strategy (8-way graph/data parallel, v2):
- Graphs are contiguous node ranges (batch is sorted). Core c owns graphs
  [8c, 8c+8) = nodes [ns_c, ne_c), and all edges whose dst lands in that
  range. Edges are grouped by (dst 128-node window, src half) and chopped
  into 128-edge tiles; per-window tiles accumulate into one PSUM bank via a
  rank-onehot scatter matmul (out[d] += onehot^T @ msgs).
- Layer 1 (1->256, rank-1 in x0) and the dynamic layer (3->64, rank-3 in
  x_dyn) collapse: per-edge messages are scalars ex*x[src]. Host stages
  x[src] AND x[dst] per edge slot (pure gathers), so pass A needs NO device
  gathers at all; all per-edge math runs as a few full-width DVE ops.
- Node phase expands aggregates to h1 via a block-diagonal PE matmul,
  applies elu, computes h2 = elu(h1) @ Ws2 and attention dot-products;
  bf16 rows [h2|s2src|pad] are AllGathered so pass B can batch-gather
  h2[src] rows with one dma_gather per (window, src-half) group.
- Pooling is a per-window matmul with a host-built (1/count) mask; the tiny
  value head runs on-device per core over its 8 graphs.

kernel(**inputs) is self-contained: host-side work is only sharding
(partition/sort/pad of indices, slicing, dtype casts) — all model math,
including derived attention constants, runs on device.
"""

import os
import numpy as np
import ml_dtypes

STAGE = int(os.environ.get("KSTAGE", "9"))
KGATHER = os.environ.get("KGATHER", "both")  # bisection: 1=prep 2=+passA 3=+node 4=+coll 5=+gathers 9=full

import concourse.bacc as bacc
import concourse.bass as bass
import concourse.mybir as mybir
import concourse.tile as tile
from concourse.masks import make_identity

F32 = mybir.dt.float32
F32R = mybir.dt.float32r
BF16 = mybir.dt.bfloat16
I32 = mybir.dt.int32
I16 = mybir.dt.int16
AF = mybir.ActivationFunctionType
OP = mybir.AluOpType

P = 128
H = 4          # heads (static encoder)
C = 64         # channels per head
HC = H * C     # 256
EPS = 1e-16
GROW = 384     # gathered H2 row width (bf16; 768B, 256B-aligned)
DROW = 128     # dstv2 row width (bf16; 256B)

NPBF = ml_dtypes.bfloat16


def brd(ap, pattern, offset=None):
    """Manual broadcast: new AP over same tensor with given [step, count] list."""
    return bass.AP(ap.tensor, ap.offset if offset is None else offset, pattern)


# ----------------------------------------------------------------------------
# Host-side sharding / planning
# ----------------------------------------------------------------------------

class Plan:
    pass


def _wrap_idx(vals):
    """[n] int array (n % 128 == 0) -> [128, n//16] i16 wrapped layout:
    index i at partition (i%16 + 16c for c in 0..7), free slot i//16."""
    n = vals.shape[0]
    arr = vals.reshape(n // 16, 16).T.astype(np.int16)  # [16, n//16]
    return np.tile(arr, (8, 1))


def host_prep(x, edge_attr, edge_index, batch, n_graphs, n_cores):
    """Pure index/layout work (sharding); no model math."""
    N = x.shape[0]
    W = n_cores
    gpc = n_graphs // W  # graphs per core
    assert gpc * W == n_graphs

    batch = np.asarray(batch).astype(np.int64)
    src = np.asarray(edge_index[0]).astype(np.int64)
    dst = np.asarray(edge_index[1]).astype(np.int64)
    ea = np.asarray(edge_attr).astype(np.float32)
    x = np.asarray(x).astype(np.float32)

    node_start = np.searchsorted(batch, np.arange(n_graphs + 1))
    core_ns = node_start[0::gpc]  # [W+1] boundaries
    nk = np.diff(core_ns)
    R = int(128 * np.ceil(nk.max() / 128))
    nwin = R // 128
    NP = W * R
    HN = NP // 2  # rows per src-half of the gathered table

    core_of = np.searchsorted(core_ns, np.arange(N), side="right") - 1
    pid = core_of * R + (np.arange(N) - core_ns[core_of])

    counts = np.bincount(batch, minlength=n_graphs).astype(np.float32)
    assert (counts > 0).all()

    dcore = np.searchsorted(core_ns, dst, side="right") - 1
    src_pid = pid[src]
    src_half = (src_pid >= HN).astype(np.int64)

    # per-core edge grouping: (window, src_half) -> padded 128-edge tiles
    per_core_pre = []
    ntiles_gw = np.zeros((W, nwin, 2), np.int64)
    for c in range(W):
        m = dcore == c
        dl = (dst[m] - core_ns[c]).astype(np.int64)
        win = dl // 128
        half = src_half[m]
        order = np.lexsort((dl, half, win))
        e_dl = dl[order]
        e_win = win[order]
        e_half = half[order]
        e_sp = src_pid[m][order]
        e_ea = ea[m][order]
        e_xs = x[src[m][order]]
        e_xd = x[dst[m][order]]
        # group boundaries over (win, half)
        gkey = e_win * 2 + e_half
        bounds = np.searchsorted(gkey, np.arange(2 * nwin + 1))
        cnt = np.diff(bounds)
        ntiles_gw[c] = np.ceil(cnt.reshape(nwin, 2) / 128).astype(np.int64)
        per_core_pre.append((e_dl, e_sp, e_ea, e_xs, e_xd, bounds))

    # shared tiling across cores (same device program for all cores)
    nt_g = ntiles_gw.max(axis=0)          # [nwin, 2]
    nt_g = np.maximum(nt_g, ntiles_gw.max(axis=0))
    # ensure each window has at least one tile overall
    for w in range(nwin):
        if nt_g[w].sum() == 0:
            nt_g[w, 0] = 1
    T = int(nt_g.sum())
    goff = np.zeros((nwin, 2), np.int64)   # tile offset of each group
    acc = 0
    for w in range(nwin):
        for hlf in range(2):
            goff[w, hlf] = acc
            acc += nt_g[w, hlf]
    assert acc == T

    per_core_arrays = []
    for c in range(W):
        e_dl, e_sp, e_ea, e_xs, e_xd, bounds = per_core_pre[c]
        mrank = np.full((T * P,), P, np.float32)
        idx_hg = np.zeros((T * P,), np.int64)
        idx_dv = np.full((T * P,), R, np.int64)
        eat = np.zeros((T * P, 2), np.float32)
        xst = np.zeros((T * P, 4), np.float32)
        xdt = np.zeros((T * P, 4), np.float32)
        for w in range(nwin):
            for hlf in range(2):
                g = 2 * w + hlf
                a, b = int(bounds[g]), int(bounds[g + 1])
                n = b - a
                if n == 0:
                    continue
                o = int(goff[w, hlf]) * P
                mrank[o : o + n] = e_dl[a:b] - 128 * w
                idx_hg[o : o + n] = e_sp[a:b] - hlf * HN
                idx_dv[o : o + n] = e_dl[a:b]
                eat[o : o + n] = e_ea[a:b]
                xst[o : o + n] = e_xs[a:b]
                xdt[o : o + n] = e_xd[a:b]

        ns, ne = int(core_ns[c]), int(core_ns[c + 1])
        pmask = np.zeros((R, gpc), np.float32)
        gidx = (batch[ns:ne] - c * gpc).astype(np.int64)
        pmask[np.arange(ne - ns), gidx] = 1.0 / counts[batch[ns:ne]]

        tileT = lambda a: np.ascontiguousarray(
            a.reshape(T, P, *a.shape[1:]).transpose(
                (1, 0) + tuple(range(2, a.ndim + 1))
            )
        )
        per_core_arrays.append(
            dict(
                m_rank=tileT(mrank).astype(NPBF),            # [128, T]
                idx_hg=_wrap_idx(idx_hg),                     # [128, T*8] i16
                idx_dv=_wrap_idx(idx_dv),                     # [128, T*8] i16
                ea_t=tileT(eat).astype(NPBF),                 # [128, T, 2]
                x_src=tileT(xst).astype(NPBF),                # [128, T, 4]
                x_dst=tileT(xdt).astype(NPBF),                # [128, T, 4]
                pmask=pmask,
            )
        )

    # block-diag expansion mask for h1 = r1t^T @ (ws1 * blk4)
    blk4 = np.zeros((H, HC), np.float32)
    for h in range(H):
        blk4[h, h * C : (h + 1) * C] = 1.0

    plan = Plan()
    plan.W = W
    plan.R = R
    plan.NP = NP
    plan.HN = HN
    plan.nwin = nwin
    plan.T = T
    plan.nt_g = nt_g
    plan.goff = goff
    plan.gpc = gpc
    plan.blk4 = blk4
    plan.tiles_per_window = tuple((int(a), int(b)) for a, b in nt_g)
    return plan, None, per_core_arrays


# ----------------------------------------------------------------------------
# Device program
# ----------------------------------------------------------------------------

def build_bass(plan):
    W, R, NP, nwin, T = plan.W, plan.R, plan.NP, plan.nwin, plan.T
    HN = plan.HN
    nt_g = plan.nt_g
    goff = plan.goff
    gpc = plan.gpc
    NTMAX = int(nt_g.max())
    NTWMAX = int(nt_g.sum(axis=1).max())

    nc = bacc.Bacc("TRN2", target_bir_lowering=False, debug=False, num_devices=W)

    def dp(name, shape, dtype=F32, out=False):
        return nc.declare_dram_parameter(name, list(shape), dtype, isOutput=out)

    xs_in = dp("x_src", [P, T, 4], BF16)
    xd_in = dp("x_dst", [P, T, 4], BF16)
    ea_in = dp("ea_t", [P, T, 2], BF16)
    mrank_in = dp("m_rank", [P, T], BF16)
    ihg_in = dp("idx_hg", [P, T * 8], I16)
    idv_in = dp("idx_dv", [P, T * 8], I16)
    pmask = dp("pmask", [R, gpc])
    blk4_in = dp("blk4", [H, HC])

    ws1 = dp("ws1", [1, HC])
    a1s = dp("a1s", [1, HC])
    a1d = dp("a1d", [1, HC])
    we1 = dp("we1", [1, 2 * HC])
    ae1 = dp("ae1", [1, HC])
    bs1 = dp("bs1", [1, HC])
    ws2 = dp("ws2", [HC, HC])
    a2s = dp("a2s", [1, HC])
    a2d = dp("a2d", [1, HC])
    we2 = dp("we2", [1, 2 * HC])
    ae2 = dp("ae2", [1, HC])
    bs2 = dp("bs2", [1, C])
    wd = dp("wd", [3, C])
    wdf = dp("wdf", [1, 3 * C])
    ads = dp("ads", [1, C])
    add_ = dp("add", [1, C])
    bd = dp("bd", [1, C])
    wv1 = dp("wv1", [C, C])
    bv1 = dp("bv1", [1, C])
    wv2 = dp("wv2", [C, 1])
    bv2 = dp("bv2", [1, 1])

    v_out = dp("v", [gpc, 1], out=True)

    # internal DRAM
    dstv2 = nc.dram_tensor("dstv2", [R + P, DROW], BF16)
    h2slice = nc.dram_tensor("h2slice", [R, GROW], BF16)
    if W > 4:
        H2ext = nc.dram_tensor("H2ext", [NP, GROW], BF16, addr_space="Shared")
    else:
        H2ext = nc.dram_tensor("H2ext", [NP, GROW], BF16)

    with tile.TileContext(nc) as tc:
        with (
            tc.tile_pool(name="const", bufs=1) as cp,
            tc.tile_pool(name="meta", bufs=1) as mp,
            tc.tile_pool(name="work", bufs=2) as wp,
            tc.tile_pool(name="gath", bufs=2) as gp,
            tc.tile_pool(name="ps", bufs=4, space="PSUM") as pp,
            tc.tile_pool(name="pst", bufs=2, space="PSUM") as pt,
        ):
            # ---------------- P0: constants -------------------------------
            ident = cp.tile([P, P], F32)
            make_identity(nc, ident[:])
            iota_mat = cp.tile([P, P], I32)
            nc.gpsimd.iota(iota_mat[:], pattern=[[1, P]], base=0, channel_multiplier=0)
            iota_f = cp.tile([P, P], F32)
            nc.vector.tensor_copy(out=iota_f[:], in_=iota_mat[:])
            iota_bf = cp.tile([P, P], BF16)
            nc.vector.tensor_copy(out=iota_bf[:], in_=iota_f[:])

            def load_row(dram, width, tag):
                t = cp.tile([1, width], F32, tag=tag)
                nc.sync.dma_start(out=t[:], in_=dram[0:1, 0:width])
                return t

            r_ws1 = load_row(ws1, HC, "r_ws1")
            r_a1s = load_row(a1s, HC, "r_a1s")
            r_a1d = load_row(a1d, HC, "r_a1d")
            r_we1 = load_row(we1, 2 * HC, "r_we1")
            r_ae1 = load_row(ae1, HC, "r_ae1")
            r_bs1 = load_row(bs1, HC, "r_bs1")
            r_a2s = load_row(a2s, HC, "r_a2s")
            r_a2d = load_row(a2d, HC, "r_a2d")
            r_we2 = load_row(we2, 2 * HC, "r_we2")
            r_ae2 = load_row(ae2, HC, "r_ae2")
            r_bs2 = load_row(bs2, C, "r_bs2")
            r_wdf = load_row(wdf, 3 * C, "r_wdf")
            r_ads = load_row(ads, C, "r_ads")
            r_add = load_row(add_, C, "r_add")
            r_bd = load_row(bd, C, "r_bd")
            r_bv1 = load_row(bv1, C, "r_bv1")
            r_bv2 = load_row(bv2, 1, "r_bv2")

            scratch = cp.tile([1, 2 * HC], F32)

            def dot_heads(out_ap, wrow, arow, nh):
                """out[0, h] = sum_c wrow[0, h*C+c] * arow[0, h*C+c]."""
                nc.vector.tensor_tensor(
                    out=scratch[0:1, 0 : nh * C], in0=wrow, in1=arow, op=OP.mult
                )
                nc.vector.reduce_sum(
                    out=out_ap,
                    in_=brd(scratch[:], [scratch[:].ap[0], [C, nh], [1, C]]),
                    axis=mybir.AxisListType.X,
                )

            # cc = [c1(4) | c1d(4)]
            cc_row = cp.tile([1, 2 * H], F32)
            dot_heads(cc_row[0:1, 0:H], r_ws1[:], r_a1s[:], H)
            dot_heads(cc_row[0:1, H : 2 * H], r_ws1[:], r_a1d[:], H)
            # M = [M1row0(4)|M1row1(4)|M2row0(4)|M2row1(4)]
            m_row = cp.tile([1, 4 * H], F32)
            dot_heads(m_row[0:1, 0:H], r_we1[0:1, 0:HC], r_ae1[:], H)
            dot_heads(m_row[0:1, H : 2 * H], r_we1[0:1, HC : 2 * HC], r_ae1[:], H)
            dot_heads(m_row[0:1, 2 * H : 3 * H], r_we2[0:1, 0:HC], r_ae2[:], H)
            dot_heads(m_row[0:1, 3 * H : 4 * H], r_we2[0:1, HC : 2 * HC], r_ae2[:], H)
            # cds = [cd(3) | cdd(3)]: cd[j] = sum_c wd[j,c]*ads[c]
            cds_row = cp.tile([1, 6], F32)
            for k, arow in ((0, r_ads), (3, r_add)):
                nc.vector.tensor_tensor(
                    out=brd(scratch[:], [scratch[:].ap[0], [C, 3], [1, C]]),
                    in0=brd(r_wdf[:], [r_wdf[:].ap[0], [C, 3], [1, C]]),
                    in1=brd(arow[:], [arow[:].ap[0], [0, 3], [1, C]]),
                    op=OP.mult,
                )
                nc.vector.reduce_sum(
                    out=cds_row[0:1, k : k + 3],
                    in_=brd(scratch[:], [scratch[:].ap[0], [C, 3], [1, C]]),
                    axis=mybir.AxisListType.X,
                )

            def prep(row_ap, width, tag, dtype=F32):
                t = cp.tile([P, width], F32, tag=tag)
                nc.gpsimd.partition_broadcast(t[:], row_ap)
                if dtype == F32:
                    return t
                tb = cp.tile([P, width], dtype, tag=tag + "_b")
                nc.vector.tensor_copy(out=tb[:], in_=t[:])
                return tb

            cc_bf = prep(cc_row[:], 2 * H, "cc_rep", BF16)
            m_bf = prep(m_row[:], 4 * H, "m_rep", BF16)
            cds_bf = prep(cds_row[:], 6, "cds_rep", BF16)
            bs1_rep = prep(r_bs1[:], HC, "bs1_rep")
            a2s_rep = prep(r_a2s[:], HC, "a2s_rep")
            a2d_rep = prep(r_a2d[:], HC, "a2d_rep")
            bs2_rep = prep(r_bs2[:], C, "bs2_rep")
            bd_rep = prep(r_bd[:], C, "bd_rep")
            bv1_rep = prep(r_bv1[:], C, "bv1_rep")
            bv2_rep = prep(r_bv2[:], 1, "bv2_rep")

            # w1blk = partition_broadcast(ws1)[0:H] * blk4  (block-diag [H, HC])
            w1full = cp.tile([P, HC], F32, tag="w1full")
            nc.gpsimd.partition_broadcast(w1full[:], r_ws1[:])
            blk4_sb = cp.tile([H, HC], F32)
            nc.sync.dma_start(out=blk4_sb[:], in_=blk4_in[:])
            w1blk = cp.tile([H, HC], F32R)
            nc.vector.tensor_tensor(
                out=w1blk[:], in0=w1full[0:H, :], in1=blk4_sb[:], op=OP.mult
            )

            ws2_sb = cp.tile([P, 2, HC], F32)  # [i_chunk][128, 256]
            nc.sync.dma_start(out=ws2_sb[:, 0, :], in_=ws2[0:P, :])
            nc.sync.dma_start(out=ws2_sb[:, 1, :], in_=ws2[P : 2 * P, :])
            ws2f = cp.tile([P, 2, HC], F32R)
            nc.vector.tensor_copy(out=ws2f[:], in_=ws2_sb[:])
            wd_sb = cp.tile([3, C], F32)
            nc.sync.dma_start(out=wd_sb[:], in_=wd[:])
            wd_f = cp.tile([3, C], F32R)
            nc.vector.tensor_copy(out=wd_f[:], in_=wd_sb[:])
            wv1_sb = cp.tile([C, C], F32)
            nc.sync.dma_start(out=wv1_sb[:], in_=wv1[:])
            wv2_sb = cp.tile([C, 1], F32)
            nc.sync.dma_start(out=wv2_sb[:], in_=wv2[:])

            # resident metadata
            mrank_sb = mp.tile([P, T], BF16)
            nc.sync.dma_start(out=mrank_sb[:], in_=mrank_in[:])
            pm_sb = mp.tile([P, nwin, gpc], F32)
            nc.sync.dma_start(
                out=pm_sb[:],
                in_=bass.AP(pmask, 0, [[gpc, P], [gpc * P, nwin], [1, gpc]]),
            )

            alE2 = mp.tile([P, T, 4], BF16)
            rhsA = mp.tile([P, T, 12], BF16)
            rA = mp.tile([P, nwin, 8], F32)
            h_sb = mp.tile([P, nwin, C], F32)

            # zero-pad rows of dstv2 (gather target for padded edge slots)
            zrow = wp.tile([P, DROW], BF16, tag="zrow")
            nc.vector.memset(zrow[:], 0.0)
            nc.sync.dma_start(out=dstv2[R : R + P, :], in_=zrow[:])

            # ---------------- P1: chunked edge prepass ---------------------
            TC = 128
            with tc.tile_pool(name="prep", bufs=2) as qp:
                for c0 in range(0, T, TC):
                    tn = min(TC, T - c0)
                    xs = qp.tile([P, TC, 4], BF16, tag="xs")
                    nc.sync.dma_start(
                        out=xs[:, 0:tn, :], in_=xs_in[:, c0 : c0 + tn, :]
                    )
                    xd = qp.tile([P, TC, 4], BF16, tag="xd")
                    nc.sync.dma_start(
                        out=xd[:, 0:tn, :], in_=xd_in[:, c0 : c0 + tn, :]
                    )
                    ea = qp.tile([P, TC, 2], BF16, tag="ea")
                    nc.sync.dma_start(
                        out=ea[:, 0:tn, :], in_=ea_in[:, c0 : c0 + tn, :]
                    )

                    al = qp.tile([P, TC, 5], F32, tag="al")
                    t4 = qp.tile([P, TC, 4], F32, tag="t4")
                    t4b = qp.tile([P, TC, 4], F32, tag="t4b")
                    xs_ap = xs[:]
                    xd_ap = xd[:]
                    ea_ap = ea[:]
                    al_ap = al[:]

                    def b1(src_ap, col, width, src_w):
                        """[P,tn,1] col of a (tn, src_w) tile -> bcast inner"""
                        return brd(
                            src_ap,
                            [src_ap.ap[0], [src_w, tn], [0, width]],
                            offset=src_ap.offset + col,
                        )

                    def c4(const_tile, lo, hi):
                        ap = const_tile[:]
                        return brd(ap, [ap.ap[0], [0, tn], [1, hi - lo]],
                                   offset=ap.offset + lo)

                    al14 = brd(al_ap, [al_ap.ap[0], [5, tn], [1, 4]])
                    # al1 = x0s*c1s + x0d*c1d + ea0*M1r0 + ea1*M1r1
                    nc.vector.tensor_tensor(
                        out=al14, in0=b1(xs_ap, 0, 4, 4), in1=c4(cc_bf, 0, 4),
                        op=OP.mult,
                    )
                    for src_ap, col, src_w, ctile, lo in (
                        (xd_ap, 0, 4, cc_bf, 4),
                        (ea_ap, 0, 2, m_bf, 0),
                        (ea_ap, 1, 2, m_bf, 4),
                    ):
                        nc.vector.tensor_tensor(
                            out=t4[:, 0:tn, :],
                            in0=b1(src_ap, col, 4, src_w),
                            in1=c4(ctile, lo, lo + 4),
                            op=OP.mult,
                        )
                        nc.vector.tensor_tensor(
                            out=al14, in0=al14, in1=t4[:, 0:tn, :], op=OP.add
                        )
                    # alE2 = ea0*M2r0 + ea1*M2r1
                    nc.vector.tensor_tensor(
                        out=t4[:, 0:tn, :], in0=b1(ea_ap, 0, 4, 2),
                        in1=c4(m_bf, 8, 12), op=OP.mult,
                    )
                    nc.vector.tensor_tensor(
                        out=t4b[:, 0:tn, :], in0=b1(ea_ap, 1, 4, 2),
                        in1=c4(m_bf, 12, 16), op=OP.mult,
                    )
                    nc.vector.tensor_tensor(
                        out=alE2[:, c0 : c0 + tn, :], in0=t4[:, 0:tn, :],
                        in1=t4b[:, 0:tn, :], op=OP.add,
                    )
                    # ald = xs_dyn . cds + xd_dyn . cdd
                    t3 = qp.tile([P, TC, 3], F32, tag="t3")
                    nc.vector.tensor_tensor(
                        out=t3[:, 0:tn, :],
                        in0=brd(xs_ap, [xs_ap.ap[0], [4, tn], [1, 3]],
                                offset=xs_ap.offset + 1),
                        in1=c4(cds_bf, 0, 3),
                        op=OP.mult,
                    )
                    ald = brd(al_ap, [al_ap.ap[0], [5, tn], [1, 1]],
                              offset=al_ap.offset + 4)
                    nc.vector.reduce_sum(
                        out=ald, in_=t3[:, 0:tn, :], axis=mybir.AxisListType.X
                    )
                    t1c = qp.tile([P, TC, 1], F32, tag="t1c")
                    nc.vector.tensor_tensor(
                        out=t3[:, 0:tn, :],
                        in0=brd(xd_ap, [xd_ap.ap[0], [4, tn], [1, 3]],
                                offset=xd_ap.offset + 1),
                        in1=c4(cds_bf, 3, 6),
                        op=OP.mult,
                    )
                    nc.vector.reduce_sum(
                        out=t1c[:, 0:tn, :], in_=t3[:, 0:tn, :],
                        axis=mybir.AxisListType.X,
                    )
                    nc.vector.tensor_tensor(
                        out=ald, in0=ald, in1=t1c[:, 0:tn, :], op=OP.add
                    )
                    # leaky relu + exp
                    t5 = qp.tile([P, TC, 5], F32, tag="t5")
                    nc.vector.tensor_scalar(
                        out=t5[:, 0:tn, :], in0=al[:, 0:tn, :],
                        scalar1=0.2, scalar2=None, op0=OP.mult,
                    )
                    nc.vector.tensor_tensor(
                        out=al[:, 0:tn, :], in0=al[:, 0:tn, :], in1=t5[:, 0:tn, :],
                        op=OP.max,
                    )
                    rhsA_ap = brd(rhsA[:], rhsA[:].ap, offset=rhsA[:].offset + c0 * 12)
                    nc.scalar.activation(
                        brd(rhsA_ap, [rhsA_ap.ap[0], [12, tn], [1, 5]]),
                        al[:, 0:tn, :],
                        AF.Exp,
                    )
                    # rhsA[5:9] = ex1 * x0s ; rhsA[9:12] = exd * xs_dyn
                    nc.vector.tensor_tensor(
                        out=brd(rhsA_ap, [rhsA_ap.ap[0], [12, tn], [1, 4]],
                                offset=rhsA_ap.offset + 5),
                        in0=brd(rhsA_ap, [rhsA_ap.ap[0], [12, tn], [1, 4]]),
                        in1=b1(xs_ap, 0, 4, 4),
                        op=OP.mult,
                    )
                    nc.vector.tensor_tensor(
                        out=brd(rhsA_ap, [rhsA_ap.ap[0], [12, tn], [1, 3]],
                                offset=rhsA_ap.offset + 9),
                        in0=brd(xs_ap, [xs_ap.ap[0], [4, tn], [1, 3]],
                                offset=xs_ap.offset + 1),
                        in1=brd(rhsA_ap, [rhsA_ap.ap[0], [12, tn], [0, 3]],
                                offset=rhsA_ap.offset + 4),
                        op=OP.mult,
                    )

            # ---------------- P2: pass A windows --------------------------
            for w in range(nwin if STAGE >= 2 else 0):
                ntw = int(nt_g[w, 0] + nt_g[w, 1])
                t0 = int(goff[w, 0])
                oh = wp.tile([P, NTWMAX, P], BF16, tag="ohA")
                nc.vector.tensor_tensor(
                    out=oh[:, 0:ntw, :],
                    in0=mrank_sb[:, t0 : t0 + ntw].to_broadcast([P, ntw, P]),
                    in1=brd(iota_bf[:], [iota_bf[:].ap[0], [0, ntw], [1, P]]),
                    op=OP.is_equal,
                )
                psA = pp.tile([P, 12], F32, tag="win", space="PSUM")
                for j in range(ntw):
                    nc.tensor.matmul(
                        out=psA[:],
                        lhsT=oh[:, j, :],
                        rhs=rhsA[:, t0 + j, :],
                        start=(j == 0),
                        stop=(j == ntw - 1),
                    )
                den = wp.tile([P, 5], F32, tag="denA")
                nc.vector.tensor_scalar(
                    out=den[:], in0=psA[:, 0:5], scalar1=EPS, scalar2=None, op0=OP.add
                )
                nc.vector.reciprocal(out=den[:], in_=den[:])
                nc.vector.tensor_tensor(
                    out=rA[:, w, 0:4], in0=psA[:, 5:9], in1=den[:, 0:4], op=OP.mult
                )
                nc.vector.tensor_tensor(
                    out=rA[:, w, 4:7],
                    in0=psA[:, 9:12],
                    in1=den[:, 4:5].to_broadcast([P, 3]),
                    op=OP.mult,
                )

            # ---------------- P3: node phase ------------------------------
            for w in range(nwin if STAGE >= 3 else 0):
                # transpose r1(4) and rd(3) -> [4,128], [3,128]
                prt = pt.tile([P, P], F32, tag="tr", space="PSUM")
                nc.tensor.transpose(
                    out=prt[0:4, :], in_=rA[:, w, 0:4], identity=ident[:]
                )
                rt = wp.tile([4, P], F32R, tag="rt")
                nc.vector.tensor_copy(out=rt[:], in_=prt[0:4, :])
                prd = pt.tile([P, P], F32, tag="tr", space="PSUM")
                nc.tensor.transpose(
                    out=prd[0:3, :], in_=rA[:, w, 4:7], identity=ident[:]
                )
                rdt = wp.tile([3, P], F32R, tag="rdt")
                nc.vector.tensor_copy(out=rdt[:], in_=prd[0:3, :])
                # h1 = r1t^T @ w1blk + bs1 ; hd = rdt^T @ wd + bd
                ph1 = pt.tile([P, HC], F32, tag="mm", space="PSUM")
                nc.tensor.matmul(
                    out=ph1[:], lhsT=rt[:], rhs=w1blk[:], start=True, stop=True
                )
                phd = pt.tile([P, C], F32, tag="mm", space="PSUM")
                nc.tensor.matmul(
                    out=phd[:], lhsT=rdt[:], rhs=wd_f[:], start=True, stop=True
                )
                nc.vector.tensor_tensor(
                    out=h_sb[:, w, :], in0=phd[:], in1=bd_rep[:], op=OP.add
                )
                h1 = wp.tile([P, HC], F32, tag="h1")
                nc.vector.tensor_tensor(out=h1[:], in0=ph1[:], in1=bs1_rep[:], op=OP.add)
                # elu
                e1 = wp.tile([P, HC], F32, tag="e1")
                nc.vector.tensor_scalar(
                    out=e1[:], in0=h1[:], scalar1=0.0, scalar2=None, op0=OP.min
                )
                nc.scalar.activation(e1[:], e1[:], AF.Exp)
                nc.vector.tensor_scalar(
                    out=e1[:], in0=e1[:], scalar1=-1.0, scalar2=None, op0=OP.add
                )
                nc.vector.tensor_scalar(
                    out=h1[:], in0=h1[:], scalar1=0.0, scalar2=None, op0=OP.max
                )
                nc.vector.tensor_tensor(out=h1[:], in0=h1[:], in1=e1[:], op=OP.add)
                # transpose h1e chunks
                h1t = wp.tile([P, 2, P], F32R, tag="h1t")
                for ch in range(2):
                    pst = pt.tile([P, P], F32, tag="tr", space="PSUM")
                    nc.tensor.transpose(
                        out=pst[:], in_=h1[:, ch * P : (ch + 1) * P], identity=ident[:]
                    )
                    nc.vector.tensor_copy(out=h1t[:, ch, :], in_=pst[:])
                ph2 = pt.tile([P, HC], F32, tag="mm", space="PSUM")
                for ch in range(2):
                    nc.tensor.matmul(
                        out=ph2[:],
                        lhsT=h1t[:, ch, :],
                        rhs=ws2f[:, ch, :],
                        start=(ch == 0),
                        stop=(ch == 1),
                    )
                # H2 row = [h2 | s2src | 0pad]; s2dst -> dstv2
                h2row = wp.tile([P, GROW], BF16, tag="h2row")
                nc.vector.tensor_copy(out=h2row[:, 0:HC], in_=ph2[:])
                nc.vector.memset(h2row[:, HC + H : GROW], 0.0)
                tm = wp.tile([P, HC], F32, tag="tm")
                s2f = wp.tile([P, 2 * H], F32, tag="s2f")
                nc.vector.tensor_tensor(out=tm[:], in0=ph2[:], in1=a2s_rep[:], op=OP.mult)
                nc.vector.reduce_sum(
                    out=s2f[:, 0:H],
                    in_=brd(tm[:], [tm[:].ap[0], [C, H], [1, C]]),
                    axis=mybir.AxisListType.X,
                )
                nc.vector.tensor_tensor(out=tm[:], in0=ph2[:], in1=a2d_rep[:], op=OP.mult)
                nc.vector.reduce_sum(
                    out=s2f[:, H : 2 * H],
                    in_=brd(tm[:], [tm[:].ap[0], [C, H], [1, C]]),
                    axis=mybir.AxisListType.X,
                )
                nc.vector.tensor_copy(out=h2row[:, HC : HC + H], in_=s2f[:, 0:H])
                sd2 = wp.tile([P, H], BF16, tag="sd2")
                nc.vector.tensor_copy(out=sd2[:], in_=s2f[:, H : 2 * H])
                nc.sync.dma_start(
                    out=dstv2[w * P : (w + 1) * P, 0:H], in_=sd2[:]
                )
                nc.sync.dma_start(out=h2slice[w * P : (w + 1) * P, :], in_=h2row[:])

            # ---------------- P4: allgather -------------------------------
            if STAGE >= 4 and STAGE != 7:
                nc.gpsimd.collective_compute(
                    "AllGather",
                    OP.bypass,
                    replica_groups=[list(range(W))],
                    ins=[h2slice[:]],
                    outs=[H2ext[:]],
                )

            # ---------------- P5: pass B ----------------------------------
            for w in range(nwin if STAGE >= 5 else 0):
                t0 = int(goff[w, 0])
                ntw = int(nt_g[w, 0] + nt_g[w, 1])
                # stream this window's wrapped index slices
                ihgw = gp.tile([P, NTWMAX * 8], I16, tag="ihgw")
                nc.sync.dma_start(
                    out=ihgw[:, 0 : ntw * 8],
                    in_=ihg_in[:, t0 * 8 : (t0 + ntw) * 8],
                )
                idvw = gp.tile([P, NTWMAX * 8], I16, tag="idvw")
                nc.sync.dma_start(
                    out=idvw[:, 0 : ntw * 8],
                    in_=idv_in[:, t0 * 8 : (t0 + ntw) * 8],
                )
                # batched gathers
                dv2g = None
                if KGATHER in ("dv2", "both"):
                    dv2g = gp.tile([P, NTWMAX, DROW], BF16, tag="dv2g")
                    nc.gpsimd.dma_gather(
                        dv2g[:, 0:ntw, :],
                        dstv2[:],
                        idvw[:, 0 : ntw * 8],
                        ntw * P,
                        ntw * P,
                        DROW,
                        single_packet=False,
                    )
                hgs = []
                for hlf in range(2):
                    ntg = int(nt_g[w, hlf])
                    if ntg == 0:
                        hgs.append(None)
                        continue
                    tg = int(goff[w, hlf])
                    if KGATHER not in ("hg", "both"):
                        hgs.append(None)
                        continue
                    hg = gp.tile([P, NTMAX, GROW], BF16, tag=f"hg{hlf}")
                    nc.gpsimd.dma_gather(
                        hg[:, 0:ntg, :],
                        H2ext[hlf * HN : (hlf + 1) * HN, :],
                        ihgw[:, (tg - t0) * 8 : (tg - t0 + ntg) * 8],
                        ntg * P,
                        ntg * P,
                        GROW,
                        single_packet=False,
                    )
                    hgs.append(hg)

                if STAGE < 6 or STAGE == 7 or KGATHER != "both":
                    continue
                psB = pp.tile([P, 4 + HC], F32, tag="win", space="PSUM")
                jw = 0
                for hlf in range(2):
                    ntg = int(nt_g[w, hlf])
                    if ntg == 0:
                        continue
                    tg = int(goff[w, hlf])
                    hg = hgs[hlf]
                    hg_ap = hg[:]
                    # al2 = s2src + s2dst + alE2 (bf16)
                    al2 = wp.tile([P, NTMAX, H], BF16, tag="al2")
                    nc.vector.tensor_tensor(
                        out=al2[:, 0:ntg, :],
                        in0=brd(hg_ap, [hg_ap.ap[0], [GROW, ntg], [1, H]],
                                offset=hg_ap.offset + HC),
                        in1=brd(dv2g[:], [dv2g[:].ap[0], [DROW, ntg], [1, H]],
                                offset=dv2g[:].offset + (tg - t0) * DROW),
                        op=OP.add,
                    )
                    nc.vector.tensor_tensor(
                        out=al2[:, 0:ntg, :],
                        in0=al2[:, 0:ntg, :],
                        in1=alE2[:, tg : tg + ntg, :],
                        op=OP.add,
                    )
                    t4h = wp.tile([P, NTMAX, H], BF16, tag="t4h")
                    nc.vector.tensor_scalar(
                        out=t4h[:, 0:ntg, :], in0=al2[:, 0:ntg, :],
                        scalar1=0.2, scalar2=None, op0=OP.mult,
                    )
                    nc.vector.tensor_tensor(
                        out=al2[:, 0:ntg, :], in0=al2[:, 0:ntg, :],
                        in1=t4h[:, 0:ntg, :], op=OP.max,
                    )
                    rhsB = wp.tile([P, NTMAX, 4 + HC], BF16, tag="rhsB")
                    rb_ap = rhsB[:]
                    nc.scalar.activation(
                        brd(rb_ap, [rb_ap.ap[0], [4 + HC, ntg], [1, H]]),
                        al2[:, 0:ntg, :],
                        AF.Exp,
                    )
                    # msgs = h2g * ex (per-head broadcast over 64 channels)
                    nc.vector.tensor_tensor(
                        out=brd(rb_ap, [rb_ap.ap[0], [4 + HC, ntg], [C, H], [1, C]],
                                offset=rb_ap.offset + H),
                        in0=brd(hg_ap, [hg_ap.ap[0], [GROW, ntg], [C, H], [1, C]]),
                        in1=brd(rb_ap, [rb_ap.ap[0], [4 + HC, ntg], [1, H], [0, C]]),
                        op=OP.mult,
                    )
                    ohB = wp.tile([P, NTMAX, P], BF16, tag="ohB")
                    nc.vector.tensor_tensor(
                        out=ohB[:, 0:ntg, :],
                        in0=mrank_sb[:, tg : tg + ntg].to_broadcast([P, ntg, P]),
                        in1=brd(iota_bf[:], [iota_bf[:].ap[0], [0, ntg], [1, P]]),
                        op=OP.is_equal,
                    )
                    for j in range(ntg):
                        nc.tensor.matmul(
                            out=psB[:],
                            lhsT=ohB[:, j, :],
                            rhs=rhsB[:, j, :],
                            start=(jw + j == 0),
                            stop=(jw + j == ntw - 1),
                        )
                    jw += ntg
                # window epilogue B
                dn2 = wp.tile([P, H], F32, tag="dn2")
                nc.vector.tensor_scalar(
                    out=dn2[:], in0=psB[:, 0:H], scalar1=EPS, scalar2=None,
                    op0=OP.add,
                )
                nc.vector.reciprocal(out=dn2[:], in_=dn2[:])
                agg = wp.tile([P, HC], F32, tag="agg")
                nc.vector.tensor_tensor(
                    out=brd(agg[:], [agg[:].ap[0], [C, H], [1, C]]),
                    in0=brd(psB[:], [psB[:].ap[0], [C, H], [1, C]],
                            offset=psB[:].offset + H),
                    in1=brd(dn2[:], [dn2[:].ap[0], [1, H], [0, C]]),
                    op=OP.mult,
                )
                # mean over heads (stride trick: inner dim = heads)
                hf = wp.tile([P, C], F32, tag="hf")
                nc.vector.reduce_sum(
                    out=hf[:],
                    in_=brd(agg[:], [agg[:].ap[0], [1, C], [C, H]]),
                    axis=mybir.AxisListType.X,
                )
                nc.vector.tensor_scalar(
                    out=hf[:], in0=hf[:], scalar1=0.25, scalar2=None, op0=OP.mult
                )
                nc.vector.tensor_tensor(out=hf[:], in0=hf[:], in1=bs2_rep[:], op=OP.add)
                nc.vector.tensor_tensor(
                    out=h_sb[:, w, :], in0=hf[:], in1=h_sb[:, w, :], op=OP.add
                )

            # ---------------- P6: pooling + value head --------------------
            pg = pp.tile([gpc, C], F32, tag="win", space="PSUM")
            for w in range(nwin):
                nc.tensor.matmul(
                    out=pg[:],
                    lhsT=pm_sb[:, w, :],
                    rhs=h_sb[:, w, :],
                    start=(w == 0),
                    stop=(w == nwin - 1),
                )
            g_sb = wp.tile([gpc, C], F32, tag="g_sb")
            nc.vector.tensor_copy(out=g_sb[:], in_=pg[:])
            pgt = pt.tile([C, gpc], F32, tag="tr", space="PSUM")
            nc.tensor.transpose(
                out=pgt[:], in_=g_sb[:], identity=ident[0:gpc, 0:gpc]
            )
            gt_sb = wp.tile([C, gpc], F32, tag="gt_sb")
            nc.vector.tensor_copy(out=gt_sb[:], in_=pgt[:])
            pv1 = pt.tile([gpc, C], F32, tag="mm", space="PSUM")
            nc.tensor.matmul(out=pv1[:], lhsT=gt_sb[:], rhs=wv1_sb[:], start=True, stop=True)
            a_sb = wp.tile([gpc, C], F32, tag="a_sb")
            nc.vector.tensor_tensor(
                out=a_sb[:], in0=pv1[:], in1=bv1_rep[0:gpc, :], op=OP.add
            )
            nc.vector.tensor_scalar(
                out=a_sb[:], in0=a_sb[:], scalar1=0.0, scalar2=None, op0=OP.max
            )
            pat = pt.tile([C, gpc], F32, tag="tr", space="PSUM")
            nc.tensor.transpose(out=pat[:], in_=a_sb[:], identity=ident[0:gpc, 0:gpc])
            at_sb = wp.tile([C, gpc], F32, tag="at_sb")
            nc.vector.tensor_copy(out=at_sb[:], in_=pat[:])
            pv2 = pt.tile([gpc, 1], F32, tag="mm", space="PSUM")
            nc.tensor.matmul(out=pv2[:], lhsT=at_sb[:], rhs=wv2_sb[:], start=True, stop=True)
            vres = wp.tile([gpc, 1], F32, tag="vres")
            nc.vector.tensor_tensor(
                out=vres[:], in0=pv2[:], in1=bv2_rep[0:gpc, :], op=OP.add
            )
            nc.sync.dma_start(out=v_out[:], in_=vres[:])

    nc.compile()
    return nc


# ----------------------------------------------------------------------------
# in_maps assembly
# ----------------------------------------------------------------------------

def make_in_maps(plan, x_pad, per_core_arrays, weights):
    w = {k: np.ascontiguousarray(v, np.float32) for k, v in weights.items()}
    shared = dict(
        ws1=w["Ws1"].reshape(1, HC),
        a1s=w["as_src1"].reshape(1, HC),
        a1d=w["as_dst1"].reshape(1, HC),
        we1=w["We1"].reshape(1, 2 * HC),
        ae1=w["ae1"].reshape(1, HC),
        bs1=w["bs1"].reshape(1, HC),
        ws2=w["Ws2"],
        a2s=w["as_src2"].reshape(1, HC),
        a2d=w["as_dst2"].reshape(1, HC),
        we2=w["We2"].reshape(1, 2 * HC),
        ae2=w["ae2"].reshape(1, HC),
        bs2=w["bs2"].reshape(1, C),
        wd=w["Wd"],
        wdf=w["Wd"].reshape(1, 3 * C),
        ads=w["ad_src"].reshape(1, C),
        add=w["ad_dst"].reshape(1, C),
        bd=w["bd"].reshape(1, C),
        wv1=w["Wv1"],
        bv1=w["bv1"].reshape(1, C),
        wv2=w["Wv2"],
        bv2=w["bv2"].reshape(1, 1),
        blk4=plan.blk4,
    )
    in_maps = []
    for c in range(plan.W):
        m = dict(shared)
        m.update(per_core_arrays[c])
        in_maps.append(m)
    return in_maps


_CACHE = {}


def kernel(**inputs):
    x = np.asarray(inputs["x"])
    edge_attr = np.asarray(inputs["edge_attr"])
    edge_index = np.asarray(inputs["edge_index"])
    batch = np.asarray(inputs["batch"])
    G = 64
    W = 8

    plan, x_pad, pca = host_prep(x, edge_attr, edge_index, batch, G, W)
    key = (plan.R, plan.T, tuple(plan.tiles_per_window))
    if key not in _CACHE:
        _CACHE[key] = build_bass(plan)
    nc = _CACHE[key]
    weights = {k: inputs[k] for k in (
        "Ws1", "as_src1", "as_dst1", "We1", "ae1", "bs1",
        "Ws2", "as_src2", "as_dst2", "We2", "ae2", "bs2",
        "Wd", "ad_src", "ad_dst", "bd", "Wv1", "bv1", "Wv2", "bv2")}
    in_maps = make_in_maps(plan, x_pad, pca, weights)
    from concourse.bass_utils import run_bass_kernel_spmd
    res = run_bass_kernel_spmd(nc, in_maps, list(range(W)))
    v = np.concatenate([res.results[c]["v"][:, 0] for c in range(W)])
    return v.astype(np.float32)
